# revision 20
# baseline (speedup 1.0000x reference)
"""AMK block kernel for 8 TRN2 NeuronCores (self-contained).

Shards: batch(4) x seq-half(2). Weights arrive host-transposed ([in,out])
and host-cast to bf16; activations flow token-major for LN and d-major
(PE-transposed) as matmul moving operands. See NOTES.md.
"""
import sys

sys.path.insert(0, "/opt/trn_rl_repo")

import numpy as np

B, N, D = 4, 2048, 768
DS = 64
INNER = 3072
NT = N // 2          # tokens per core
DC = D // 128        # 6 d-chunks
IC = INNER // 128    # 24 inner chunks
TC = NT // 128       # 8 token chunks
RSH = (D * DS) // 8  # 6144 hyper rows per core
EPS = 1e-5

_CACHE = {}
TRACE = False
_LAST_RESULT = None


def _build(sp_val, use_ln1_gb, use_ln2_gb, use_mpb):
    from concourse import bacc, tile, mybir
    from concourse import tile_utils
    # 192KiB is a stale cap; cayman has 224 phys / 208 usable per partition
    tile_utils.max_sbuf_usage = 208 * 1024

    f32 = mybir.dt.float32
    f32r = mybir.dt.float32r
    bf16 = mybir.dt.bfloat16
    AF = mybir.ActivationFunctionType
    ALU = mybir.AluOpType
    AX = mybir.AxisListType

    nc = bacc.Bacc(None, target_bir_lowering=False)

    # ---------------- I/O ----------------
    q_in = nc.dram_tensor("q_in", [NT, D], f32, kind="ExternalInput")
    x_in = nc.dram_tensor("x_in", [NT, D], f32, kind="ExternalInput")
    # host-transposed (and bf16-cast) weights:
    hqwt = nc.dram_tensor("hqwt", [D, RSH], bf16, kind="ExternalInput")
    hkwt = nc.dram_tensor("hkwt", [D, RSH], bf16, kind="ExternalInput")
    wupt = nc.dram_tensor("wupt", [48, D, 128], bf16, kind="ExternalInput")
    wdnt = nc.dram_tensor("wdnt", [DC, INNER, 128], bf16, kind="ExternalInput")
    wmpt = nc.dram_tensor("wmpt", [D, D], bf16, kind="ExternalInput")
    dw = nc.dram_tensor("dw", [128, IC, 3], f32, kind="ExternalInput")
    bqk = nc.dram_tensor("bqk", [DS, 2], f32, kind="ExternalInput")
    bhot = nc.dram_tensor("bhot", [128, 4], f32, kind="ExternalInput")
    hmask = nc.dram_tensor("hmask", [128, 2], f32, kind="ExternalInput")
    rows = nc.dram_tensor("rows", [1, 5 * D], f32, kind="ExternalInput")
    out_e = nc.dram_tensor("out", [NT, D], f32, kind="ExternalOutput")

    ident_f32_d = nc.inline_tensor(np.eye(128, dtype=np.float32), name="idf32")
    import ml_dtypes
    ident_bf16_d = nc.inline_tensor(
        np.eye(128).astype(ml_dtypes.bfloat16), name="idbf16"
    )

    # collective bounce buffers + scratch (internal DRAM)
    qp_i = nc.dram_tensor("qp_i", [D, 4], f32)
    qp_o = nc.dram_tensor("qp_o", [D, 4], f32, addr_space="Shared")
    og_i = nc.dram_tensor("og_i", [RSH, 8], f32)
    og_o = nc.dram_tensor("og_o", [8 * RSH, 8], f32, addr_space="Shared")
    ce_i = nc.dram_tensor("ce_i", [DS, 772], f32)
    ce_o = nc.dram_tensor("ce_o", [DS, 772], f32)
    hx_i = nc.dram_tensor("hx_i", [2, INNER], f32)
    hx_o = nc.dram_tensor("hx_o", [4, INNER], f32)
    qint_d = nc.dram_tensor("qint_d", [NT, D], f32)
    hf_d = nc.dram_tensor("hf_d", [INNER, NT], bf16)

    RG_ALL = [list(range(8))]
    RG_PAIR = [[0, 1], [2, 3], [4, 5], [6, 7]]

    with tile.TileContext(nc) as tc:
        import contextlib

        ctx = contextlib.ExitStack()
        with ctx:
            cst = ctx.enter_context(tc.tile_pool(name="cst", bufs=1))
            big_a = ctx.enter_context(tc.tile_pool(name="big_a", bufs=1))
            big_b = ctx.enter_context(tc.tile_pool(name="big_b", bufs=1))
            med = ctx.enter_context(tc.tile_pool(name="med", bufs=1))
            sml = ctx.enter_context(tc.tile_pool(name="sml", bufs=2))
            scr = ctx.enter_context(tc.tile_pool(name="scr", bufs=3))
            wst = ctx.enter_context(tc.tile_pool(name="wst", bufs=2))
            ps = ctx.enter_context(tc.tile_pool(name="ps", bufs=1, space="PSUM"))
            ps_tp = ctx.enter_context(
                tc.tile_pool(name="ps_tp", bufs=2, space="PSUM")
            )
            ps_acc = ctx.enter_context(
                tc.tile_pool(name="ps_acc", bufs=1, space="PSUM")
            )

            # ---- constants to SBUF ----
            id32 = cst.tile([128, 128], f32, tag="id32")
            nc.sync.dma_start(id32[:], ident_f32_d[:])
            id16 = cst.tile([128, 128], bf16, tag="id16")
            nc.sync.dma_start(id16[:], ident_bf16_d[:])
            dw_sb = cst.tile([128, IC, 3], f32, tag="dw")
            nc.sync.dma_start(dw_sb[:], dw[:])
            bqk_sb = cst.tile([DS, 2], f32, tag="bqk")
            nc.sync.dma_start(bqk_sb[:], bqk[:])
            bhot_sb = cst.tile([128, 4], f32, tag="bhot")
            nc.sync.dma_start(bhot_sb[:], bhot[:])
            hmask_sb = cst.tile([128, 2], f32, tag="hmask")
            nc.sync.dma_start(hmask_sb[:], hmask[:])
            rows_sb = cst.tile([1, 5 * D], f32, tag="rows")
            nc.sync.dma_start(rows_sb[:], rows[:])
            zero1 = cst.tile([128, 1], f32, tag="zero1")
            nc.vector.memset(zero1[:], 0.0)
            eps1 = cst.tile([128, 1], f32, tag="eps1")
            nc.vector.memset(eps1[:], EPS)
            nc.const_aps.aps[(f32, 0.0)] = zero1[:]
            nc.const_aps.aps[(f32, EPS)] = eps1[:]

            def row_bc(i):  # [1,768] -> broadcast [128,768]
                return rows_sb[0:1, i * D:(i + 1) * D].partition_broadcast(128)

            # =========================================================
            # Phase A: LN1 + H (token-major bf16), transpose -> H_d, q_pool
            # =========================================================
            h_tok = big_a.tile([128, TC, D], bf16, tag="bigA")
            h_d = big_b.tile([128, DC, NT], bf16, tag="h_d")
            qs2 = sml.tile([128, DC, 2], f32, tag="qs2")

            def layernorm_chunk(src_ap, out_ap, gcol, bcol, use_gb, extra_add):
                """LN over free dim of [128,768] chunk; out = LN*g+b (+extra)."""
                mv6 = scr.tile([128, 2, 6], f32, tag="mv6")
                for g in range(2):
                    nc.vector.bn_stats(
                        mv6[:, g, :], src_ap[:, g * 384:(g + 1) * 384]
                    )
                mv = scr.tile([128, 2], f32, tag="mv")
                nc.vector.bn_aggr(mv[:], mv6[:])
                rs = scr.tile([128, 1], f32, tag="rs")
                sd = scr.tile([128, 1], f32, tag="sd")
                nc.scalar.activation(sd[:], mv[:, 1:2], AF.Sqrt, bias=EPS)
                nc.vector.reciprocal(rs[:], sd[:])
                xn = scr.tile([128, D], f32, tag="t768")
                nc.vector.tensor_scalar(
                    xn[:], src_ap, mv[:, 0:1], rs[:, 0:1], ALU.subtract, ALU.mult
                )
                if use_gb:
                    nc.vector.tensor_tensor(xn[:], xn[:], row_bc(gcol), ALU.mult)
                    nc.vector.tensor_tensor(xn[:], xn[:], row_bc(bcol), ALU.add)
                if extra_add is not None:
                    nc.vector.tensor_tensor(out_ap, xn[:], extra_add, ALU.add)
                else:
                    nc.vector.tensor_copy(out_ap, xn[:])

            for t in range(TC):
                qc = sml.tile([128, D], f32, tag="qc")
                nc.sync.dma_start(qc[:], q_in[t * 128:(t + 1) * 128, :])
                xc = sml.tile([128, D], f32, tag="xc")
                nc.sync.dma_start(xc[:], x_in[t * 128:(t + 1) * 128, :])
                layernorm_chunk(qc[:], h_tok[:, t, :], 0, 1, use_ln1_gb, xc[:])

            # transpose H -> H_d (bf16) + q_pool partials via accum_out
            for dc in range(DC):
                for tq in range(2):
                    ptp = ps_tp.tile([128, 512], bf16, tag="tp")
                    for k in range(4):
                        t = tq * 4 + k
                        nc.tensor.matmul(
                            ptp[:, k * 128:(k + 1) * 128],
                            h_tok[:, t, dc * 128:(dc + 1) * 128],
                            id16[:],
                            is_transpose=True,
                        )
                    nc.scalar.activation(
                        h_d[:, dc, tq * 512:(tq + 1) * 512],
                        ptp[:],
                        AF.Copy,
                        accum_out=qs2[:, dc, tq:tq + 1],
                    )

            # qp contribution: [128, DC, 4] = bhot * (qs2.sum) / N
            qp_sb = sml.tile([128, DC, 4], f32, tag="qp_sb")
            qsum = sml.tile([128, DC], f32, tag="qsum")
            nc.vector.tensor_tensor(qsum[:], qs2[:, :, 0], qs2[:, :, 1], ALU.add)
            for dc in range(DC):
                nc.vector.tensor_scalar(
                    qp_sb[:, dc, :],
                    bhot_sb[:],
                    qsum[:, dc:dc + 1],
                    1.0 / N,
                    ALU.mult,
                    ALU.mult,
                )
            nc.gpsimd.dma_start(
                qp_i[:].rearrange("(c p) b -> p c b", p=128), qp_sb[:]
            )
            nc.gpsimd.collective_compute(
                "AllReduce", ALU.add, replica_groups=RG_ALL,
                ins=[qp_i[:]], outs=[qp_o[:]],
            )
            qp_all = sml.tile([128, DC, 4], f32, tag="qp_all")
            nc.gpsimd.dma_start(
                qp_all[:], qp_o[:].rearrange("(c p) b -> p c b", p=128)
            )
            qp_bf = sml.tile([128, DC, 4], bf16, tag="qp_bf")
            nc.vector.tensor_copy(qp_bf[:], qp_all[:])

            # =========================================================
            # Phase B: hyper O_part [RSH, 8] -> AllGather -> Omega (bf16)
            # =========================================================
            for m, wsrc in ((0, hqwt), (1, hkwt)):
                for rb in range(RSH // 256):
                    wblk = wst.tile([128, DC, 256], bf16, tag="hypW")
                    nc.sync.dma_start(
                        wblk[:],
                        wsrc[:, rb * 256:(rb + 1) * 256]
                        .rearrange("(c p) r -> p c r", p=128),
                    )
                    po = ps_acc.tile([4, 256], f32, tag="acc")
                    for j in range(DC):
                        nc.tensor.matmul(
                            po[:],
                            qp_bf[:, j, :],
                            wblk[:, j, :],
                            start=(j == 0),
                            stop=(j == DC - 1),
                        )
                    ob = scr.tile([4, 256], f32, tag="og_blk")
                    nc.scalar.activation(ob[:], po[:], AF.Copy)
                    nc.sync.dma_start(
                        og_i[rb * 256:(rb + 1) * 256, m * 4:(m + 1) * 4]
                        .rearrange("r c -> c r"),
                        ob[:],
                    )
            nc.gpsimd.collective_compute(
                "AllGather", ALU.bypass, replica_groups=RG_ALL,
                ins=[og_i[:]], outs=[og_o[:]],
            )
            # Omega: mask-reduce gathered columns by bhot (SPMD-safe), bf16
            om = []
            for m in range(2):
                omt = med.tile([128, DC, DS], bf16, tag=f"om{m}")
                for dc in range(DC):
                    og4 = scr.tile([128, DS, 4], f32, tag="og4")
                    nc.sync.dma_start(
                        og4[:],
                        og_o[:].rearrange(
                            "(c p s) e -> p c s e", p=128, s=DS
                        )[:, dc, :, m * 4:(m + 1) * 4],
                    )
                    acc4 = scr.tile([128, DS], f32, tag="omacc")
                    t4 = scr.tile([128, DS], f32, tag="omtmp")
                    nc.vector.tensor_scalar(
                        acc4[:], og4[:, :, 0], bhot_sb[:, 0:1], None, ALU.mult
                    )
                    for bb in range(1, 4):
                        nc.vector.tensor_scalar(
                            t4[:], og4[:, :, bb], bhot_sb[:, bb:bb + 1], None,
                            ALU.mult,
                        )
                        nc.vector.tensor_tensor(acc4[:], acc4[:], t4[:], ALU.add)
                    nc.vector.tensor_copy(omt[:, dc, :], acc4[:])
                om.append(omt)

            # =========================================================
            # Phase C: attention  Phi -> C -> Attraction -> m -> m_proj
            # =========================================================
            phi = []
            for m in range(2):
                phi_t = med.tile([DS, NT], bf16, tag=f"phi{m}")
                for tq in range(2):
                    pph = ps_acc.tile([DS, 512], f32, tag="acc")
                    for dc in range(DC):
                        nc.tensor.matmul(
                            pph[:],
                            om[m][:, dc, :],
                            h_d[:, dc, tq * 512:(tq + 1) * 512],
                            start=(dc == 0),
                            stop=(dc == DC - 1),
                        )
                    # elu(x+b)+1 = exp(u - relu(u)) + relu(u),  u = x + bias
                    rl = sml.tile([DS, 512], f32, tag="rl")
                    nc.scalar.activation(
                        rl[:], pph[:], AF.Relu, bias=bqk_sb[:, m:m + 1]
                    )
                    u = sml.tile([DS, 512], f32, tag="u")
                    nc.vector.tensor_scalar(
                        u[:], pph[:], bqk_sb[:, m:m + 1], None, ALU.add
                    )
                    nc.vector.tensor_tensor(u[:], u[:], rl[:], ALU.subtract)
                    ex = sml.tile([DS, 512], f32, tag="ex")
                    nc.scalar.activation(ex[:], u[:], AF.Exp)
                    nc.vector.tensor_tensor(
                        phi_t[:, tq * 512:(tq + 1) * 512], ex[:], rl[:], ALU.add
                    )
                phi.append(phi_t)

            # phi_k_sum [DS,1]
            pks = sml.tile([DS, 1], f32, tag="pks")
            nc.vector.tensor_reduce(pks[:], phi[1][:], AX.X, ALU.add)

            # Phi_K token-major bf16 [128, TC, DS]
            pk_tok = med.tile([128, TC, DS], bf16, tag="pk_tok")
            for tq in range(2):
                ptp = ps_tp.tile([128, 256], bf16, tag="tp")
                for k in range(4):
                    t = tq * 4 + k
                    nc.tensor.matmul(
                        ptp[:, k * DS:(k + 1) * DS],
                        phi[1][:, t * 128:(t + 1) * 128],
                        id16[:DS, :DS],
                        is_transpose=True,
                    )
                nc.scalar.activation(
                    pk_tok[:, tq * 4:(tq + 1) * 4, :]
                    .rearrange("p t s -> p (t s)"),
                    ptp[:],
                    AF.Copy,
                )

            # C partial [DS, 772]: cols 0:768 = C, 768 = pks
            ce_sb = med.tile([DS, 772], f32, tag="ce_sb")
            for off, w in ((0, 512), (512, 256)):
                pc0 = ps_acc.tile([DS, w], f32, tag="acc")
                for t in range(TC):
                    nc.tensor.matmul(
                        pc0[:],
                        pk_tok[:, t, :],
                        h_tok[:, t, off:off + w],
                        start=(t == 0),
                        stop=(t == TC - 1),
                    )
                nc.scalar.activation(ce_sb[:, off:off + w], pc0[:], AF.Copy)
            nc.vector.tensor_copy(ce_sb[:, 768:769], pks[:])
            nc.vector.memset(ce_sb[:, 769:772], 0.0)
            nc.gpsimd.dma_start(ce_i[:], ce_sb[:])
            nc.gpsimd.collective_compute(
                "AllReduce", ALU.add, replica_groups=RG_PAIR,
                ins=[ce_i[:]], outs=[ce_o[:]],
            )
            ce_all = med.tile([DS, 772], bf16, tag="ce_a16")
            nc.gpsimd.dma_start(ce_all[:], ce_o[:])

            # Attraction + m (token-major), m transposed -> m_d (bf16)
            m_d = med.tile([128, DC, NT], bf16, tag="mdq")
            for t in range(TC):
                pat = ps_acc.tile([128, 772], f32, tag="acc")
                nc.tensor.matmul(
                    pat[:, 0:512],
                    phi[0][:, t * 128:(t + 1) * 128],
                    ce_all[:, 0:512],
                )
                nc.tensor.matmul(
                    pat[:, 512:772],
                    phi[0][:, t * 128:(t + 1) * 128],
                    ce_all[:, 512:772],
                )
                dn = scr.tile([128, 1], f32, tag="dn")
                nc.scalar.activation(dn[:], pat[:, 768:769], AF.Abs)
                nc.vector.tensor_scalar(dn[:], dn[:], 1.0, None, ALU.add)
                rn = scr.tile([128, 1], f32, tag="rn")
                nc.vector.reciprocal(rn[:], dn[:])
                mm = scr.tile([128, D], f32, tag="t768")
                nc.vector.tensor_scalar(
                    mm[:], pat[:, 0:768], rn[:, 0:1], None, ALU.mult
                )
                nc.vector.tensor_tensor(mm[:], mm[:], h_tok[:, t, :], ALU.subtract)
                for dc in range(DC):
                    ptpm = ps_tp.tile([128, 128], f32, tag="tp")
                    nc.tensor.matmul(
                        ptpm[:],
                        mm[:, dc * 128:(dc + 1) * 128],
                        id32[:],
                        is_transpose=True,
                    )
                    nc.scalar.activation(
                        m_d[:, dc, t * 128:(t + 1) * 128], ptpm[:], AF.Copy
                    )

            # m_proj weights (bf16, [in,out], resident)
            wmp_sb = med.tile([128, DC, D], bf16, tag="wmpT")
            nc.sync.dma_start(
                wmp_sb[:], wmpt[:].rearrange("(c p) o -> p c o", p=128)
            )

            # m_proj -> Q_interact (-> DRAM) -> LN2 -> qn2_d (bf16)
            qn2_d = med.tile([128, DC, NT], bf16, tag="mdq2")
            for t in range(TC):
                pmp = ps_acc.tile([128, 768], f32, tag="acc")
                for half, w in ((0, 512), (1, 256)):
                    for dc in range(DC):
                        nc.tensor.matmul(
                            pmp[:, half * 512:half * 512 + w],
                            m_d[:, dc, t * 128:(t + 1) * 128],
                            wmp_sb[:, dc, half * 512:half * 512 + w],
                            start=(dc == 0),
                            stop=(dc == DC - 1),
                        )
                qi = scr.tile([128, D], f32, tag="t768")
                nc.scalar.activation(qi[:], pmp[:], AF.Copy, scale=sp_val)
                qc2 = scr.tile([128, D], f32, tag="t768")
                nc.sync.dma_start(qc2[:], q_in[t * 128:(t + 1) * 128, :])
                nc.vector.tensor_tensor(qi[:], qi[:], qc2[:], ALU.add)
                if use_mpb:
                    nc.vector.tensor_tensor(qi[:], qi[:], row_bc(4), ALU.add)
                nc.sync.dma_start(qint_d[t * 128:(t + 1) * 128, :], qi[:])
                qn = scr.tile([128, D], f32, tag="t768")
                layernorm_chunk(qi[:], qn[:], 2, 3, use_ln2_gb, None)
                for dc in range(DC):
                    ptpm = ps_tp.tile([128, 128], f32, tag="tp")
                    nc.tensor.matmul(
                        ptpm[:],
                        qn[:, dc * 128:(dc + 1) * 128],
                        id32[:],
                        is_transpose=True,
                    )
                    nc.scalar.activation(
                        qn2_d[:, dc, t * 128:(t + 1) * 128], ptpm[:], AF.Copy
                    )

            # =========================================================
            # Phase D: FFN  GU -> silu*U -> hf (DRAM) -> conv -> W_down
            # =========================================================
            hx_sb = sml.tile([128, 2, IC], f32, tag="hx_sb")
            for ic in range(IC):
                pg = ps.tile([128, NT], f32, tag="gup")
                pu = ps.tile([128, NT], f32, tag="guu")
                for oc, pdst in ((ic, pg), (ic + IC, pu)):
                    wt = wst.tile([128, DC, 128], bf16, tag="wupT")
                    nc.sync.dma_start(
                        wt[:], wupt[oc].rearrange("(c p) o -> p c o", p=128)
                    )
                    for tq in range(2):
                        for dc in range(DC):
                            nc.tensor.matmul(
                                pdst[:, tq * 512:(tq + 1) * 512],
                                wt[:, dc, :],
                                qn2_d[:, dc, tq * 512:(tq + 1) * 512],
                                start=(dc == 0),
                                stop=(dc == DC - 1),
                            )
                # hf = silu(G)*U -> DRAM; collect boundary cols
                hf = sml.tile([128, NT], bf16, tag="hf")
                nc.scalar.activation(hf[:], pg[:], AF.Silu)
                nc.vector.tensor_tensor(hf[:], hf[:], pu[:], ALU.mult)
                nc.vector.tensor_copy(hx_sb[:, 0, ic:ic + 1], hf[:, 0:1])
                nc.vector.tensor_copy(hx_sb[:, 1, ic:ic + 1], hf[:, NT - 1:NT])
                nc.sync.dma_start(hf_d[ic * 128:(ic + 1) * 128, :], hf[:])

            # halo exchange (pair AllGather)
            nc.gpsimd.dma_start(
                hx_i[:].rearrange("e (c p) -> p e c", p=128), hx_sb[:]
            )
            nc.gpsimd.collective_compute(
                "AllGather", ALU.bypass, replica_groups=RG_PAIR,
                ins=[hx_i[:]], outs=[hx_o[:]],
            )
            hxn = sml.tile([128, 2, IC], f32, tag="hxn")
            nc.gpsimd.dma_start(
                hxn[:], hx_o[1:3, :].rearrange("e (c p) -> p e c", p=128)
            )
            halo = sml.tile([128, IC, 2], f32, tag="halo")
            for k in range(2):
                nc.vector.tensor_scalar(
                    halo[:, :, k], hxn[:, k, :], hmask_sb[:, k:k + 1], None,
                    ALU.mult,
                )

            # conv (reload hf with halo) -> co (SBUF, bf16, full)
            co = big_a.tile([128, IC, NT], bf16, tag="bigA")
            for ic in range(IC):
                hfr = sml.tile([128, NT + 2], bf16, tag="hfr")
                nc.sync.dma_start(
                    hfr[:, 1:NT + 1], hf_d[ic * 128:(ic + 1) * 128, :]
                )
                nc.vector.tensor_copy(hfr[:, 0:1], halo[:, ic, 0:1])
                nc.vector.tensor_copy(hfr[:, NT + 1:NT + 2], halo[:, ic, 1:2])
                t0 = sml.tile([128, NT], bf16, tag="cv0")
                nc.vector.tensor_scalar(
                    t0[:], hfr[:, 0:NT], dw_sb[:, ic, 0:1], None, ALU.mult
                )
                t1 = sml.tile([128, NT], bf16, tag="cv1")
                nc.vector.tensor_scalar(
                    t1[:], hfr[:, 1:NT + 1], dw_sb[:, ic, 1:2], None, ALU.mult
                )
                nc.vector.tensor_tensor(t0[:], t0[:], t1[:], ALU.add)
                t2 = sml.tile([128, NT], bf16, tag="cv1")
                nc.vector.tensor_scalar(
                    t2[:], hfr[:, 2:NT + 2], dw_sb[:, ic, 2:3], None, ALU.mult
                )
                nc.vector.tensor_tensor(co[:, ic, :], t0[:], t2[:], ALU.add)

            # W_down: H_out (d-major) -> transpose -> + Q_int -> store
            ho_d = med.tile([128, DC, NT], bf16, tag="mdq")
            for oc in range(DC):
                wd = wst.tile([128, IC, 128], bf16, tag="wdnT")
                nc.sync.dma_start(
                    wd[:], wdnt[oc].rearrange("(c p) o -> p c o", p=128)
                )
                pho = ps_acc.tile([128, NT], f32, tag="acc")
                for tq in range(2):
                    for ic in range(IC):
                        nc.tensor.matmul(
                            pho[:, tq * 512:(tq + 1) * 512],
                            wd[:, ic, :],
                            co[:, ic, tq * 512:(tq + 1) * 512],
                            start=(ic == 0),
                            stop=(ic == IC - 1),
                        )
                nc.scalar.activation(ho_d[:, oc, :], pho[:], AF.Copy)
            for t in range(TC):
                ptp = ps_tp.tile([128, 768], bf16, tag="tp")
                for dc in range(DC):
                    nc.tensor.matmul(
                        ptp[:, dc * 128:(dc + 1) * 128],
                        ho_d[:, dc, t * 128:(t + 1) * 128],
                        id16[:],
                        is_transpose=True,
                    )
                qr = scr.tile([128, D], f32, tag="t768")
                nc.sync.dma_start(qr[:], qint_d[t * 128:(t + 1) * 128, :])
                ot = scr.tile([128, D], f32, tag="t768")
                nc.vector.tensor_tensor(ot[:], ptp[:], qr[:], ALU.add)
                nc.sync.dma_start(out_e[t * 128:(t + 1) * 128, :], ot[:])

    nc.finalize()
    return nc


def _install_trace_hook():
    import types
    import antenv
    if "antenv.axon_hooks" in sys.modules:
        return
    mod = types.ModuleType("antenv.axon_hooks")
    _h = [None]
    mod.set_axon_ntff_profile_hook = lambda h: _h.__setitem__(0, h)
    mod.get_axon_ntff_profile_hook = lambda: _h[0]
    sys.modules["antenv.axon_hooks"] = mod
    antenv.axon_hooks = mod
    try:
        from trn_agent_boot.trn_boot import _ntff_profile_via_ctypes
        mod.set_axon_ntff_profile_hook(
            _ntff_profile_via_ctypes("/opt/axon/libaxon_pjrt.so")
        )
    except Exception as e:
        print(f"ntff hook install failed: {e}")


def kernel(**inputs):
    from concourse.bass_utils import run_bass_kernel_spmd
    import ml_dtypes

    if TRACE:
        _install_trace_hook()

    bfnp = ml_dtypes.bfloat16

    Q_in = np.asarray(inputs["Q_in"], np.float32)
    X = np.asarray(inputs["X"], np.float32)
    dt = float(np.asarray(inputs["dt"]))
    hyper_q_w = np.asarray(inputs["hyper_q_w"], np.float32)
    hyper_k_w = np.asarray(inputs["hyper_k_w"], np.float32)
    B_Q = np.asarray(inputs["B_Q"], np.float32)
    B_K = np.asarray(inputs["B_K"], np.float32)
    m_proj_w = np.asarray(inputs["m_proj_w"], np.float32)
    m_proj_b = np.asarray(inputs["m_proj_b"], np.float32)
    n1g = np.asarray(inputs["norm1_g"], np.float32)
    n1b = np.asarray(inputs["norm1_b"], np.float32)
    n2g = np.asarray(inputs["norm2_g"], np.float32)
    n2b = np.asarray(inputs["norm2_b"], np.float32)
    W_up_w = np.asarray(inputs["W_up_w"], np.float32)
    dw_w = np.asarray(inputs["dw_w"], np.float32)
    W_down_w = np.asarray(inputs["W_down_w"], np.float32)

    sp = float(np.log1p(np.exp(dt)))
    use_ln1_gb = not (np.allclose(n1g, 1.0) and np.allclose(n1b, 0.0))
    use_ln2_gb = not (np.allclose(n2g, 1.0) and np.allclose(n2b, 0.0))
    use_mpb = not np.allclose(m_proj_b, 0.0)

    key = (sp, use_ln1_gb, use_ln2_gb, use_mpb)
    if key not in _CACHE:
        _CACHE[key] = _build(sp, use_ln1_gb, use_ln2_gb, use_mpb)
    nc = _CACHE[key]

    # host-side weight prep (transpose + bf16 cast + tiling)
    wupt_h = np.ascontiguousarray(
        W_up_w.T.astype(bfnp).reshape(D, 48, 128).transpose(1, 0, 2)
    )
    wdnt_h = np.ascontiguousarray(
        W_down_w.T.astype(bfnp).reshape(INNER, DC, 128).transpose(1, 0, 2)
    )
    wmpt_h = np.ascontiguousarray(m_proj_w.T.astype(bfnp))
    hqwt_h = np.ascontiguousarray(hyper_q_w.astype(bfnp).T)
    hkwt_h = np.ascontiguousarray(hyper_k_w.astype(bfnp).T)

    dw_dev = np.ascontiguousarray(
        dw_w[:, 0, :].reshape(IC, 128, 3).transpose(1, 0, 2)
    )
    bqk_dev = np.stack([B_Q, B_K], axis=1)
    rows_dev = np.ascontiguousarray(
        np.concatenate([n1g, n1b, n2g, n2b, sp * m_proj_b])[None, :]
    )

    in_maps = []
    for c in range(8):
        b, h = divmod(c, 2)
        bhot_dev = np.zeros((128, 4), np.float32)
        bhot_dev[:, b] = 1.0
        hmask_dev = np.zeros((128, 2), np.float32)
        hmask_dev[:, 0] = 1.0 if h == 1 else 0.0
        hmask_dev[:, 1] = 1.0 if h == 0 else 0.0
        in_maps.append({
            "q_in": np.ascontiguousarray(Q_in[b, h * NT:(h + 1) * NT]),
            "x_in": np.ascontiguousarray(X[b, h * NT:(h + 1) * NT]),
            "hqwt": np.ascontiguousarray(hqwt_h[:, c * RSH:(c + 1) * RSH]),
            "hkwt": np.ascontiguousarray(hkwt_h[:, c * RSH:(c + 1) * RSH]),
            "wupt": wupt_h,
            "wdnt": wdnt_h,
            "wmpt": wmpt_h,
            "dw": dw_dev,
            "bqk": bqk_dev,
            "bhot": bhot_dev,
            "hmask": hmask_dev,
            "rows": rows_dev,
        })

    global _LAST_RESULT
    res = run_bass_kernel_spmd(nc, in_maps, list(range(8)), trace=TRACE)
    _LAST_RESULT = res
    out = np.empty((B, N, D), np.float32)
    for c in range(8):
        b, h = divmod(c, 2)
        out[b, h * NT:(h + 1) * NT] = res.results[c]["out"]
    return out


if __name__ == "__main__":
    rng = np.random.default_rng(0)
    ins = {
        "Q_in": rng.normal(size=(B, N, D)).astype(np.float32),
        "X": rng.normal(size=(B, N, D)).astype(np.float32),
        "dt": np.float32(0.1),
        "hyper_q_w": rng.normal(size=(D * DS, D)).astype(np.float32) / 27.7,
        "hyper_k_w": rng.normal(size=(D * DS, D)).astype(np.float32) / 27.7,
        "B_Q": rng.normal(size=(DS,)).astype(np.float32) * 0.02,
        "B_K": rng.normal(size=(DS,)).astype(np.float32) * 0.02,
        "m_proj_w": rng.normal(size=(D, D)).astype(np.float32) * 0.02,
        "m_proj_b": np.zeros((D,), np.float32),
        "norm1_g": np.ones((D,), np.float32),
        "norm1_b": np.zeros((D,), np.float32),
        "norm2_g": np.ones((D,), np.float32),
        "norm2_b": np.zeros((D,), np.float32),
        "W_up_w": rng.normal(size=(2 * INNER, D)).astype(np.float32) / 27.7,
        "dw_w": rng.normal(size=(INNER, 1, 3)).astype(np.float32) / 1.7,
        "W_down_w": rng.normal(size=(D, INNER)).astype(np.float32) / 55.4,
    }
    out = kernel(**ins)
    print("out", out.shape, out.dtype, np.abs(out).mean())


# revision 24
# speedup vs baseline: 1.6485x; 1.6485x over previous
"""AMK block kernel for 8 TRN2 NeuronCores (self-contained).

Shards: batch(4) x seq-half(2). Weights arrive host-transposed ([in,out])
and host-cast to bf16; activations flow token-major for LN and d-major
(PE-transposed) as matmul moving operands. See NOTES.md.
"""
import sys

sys.path.insert(0, "/opt/trn_rl_repo")

import numpy as np

B, N, D = 4, 2048, 768
DS = 64
INNER = 3072
NT = N // 2          # tokens per core
DC = D // 128        # 6 d-chunks
IC = INNER // 128    # 24 inner chunks
TC = NT // 128       # 8 token chunks
RSH = (D * DS) // 8  # 6144 hyper rows per core
EPS = 1e-5

_CACHE = {}
TRACE = False
_LAST_RESULT = None


def _build(sp_val, use_ln1_gb, use_ln2_gb, use_mpb):
    from concourse import bacc, tile, mybir
    from concourse import tile_utils
    # 192KiB is a stale cap; cayman has 224 phys / 208 usable per partition
    tile_utils.max_sbuf_usage = 208 * 1024

    f32 = mybir.dt.float32
    f32r = mybir.dt.float32r
    bf16 = mybir.dt.bfloat16
    AF = mybir.ActivationFunctionType
    ALU = mybir.AluOpType
    AX = mybir.AxisListType

    nc = bacc.Bacc(None, target_bir_lowering=False)

    # ---------------- I/O ----------------
    q_in = nc.dram_tensor("q_in", [NT, D], f32, kind="ExternalInput")
    x_in = nc.dram_tensor("x_in", [NT, D], f32, kind="ExternalInput")
    # host-transposed (and bf16-cast) weights:
    hqwt = nc.dram_tensor("hqwt", [24, 128, DC, 256], bf16, kind="ExternalInput")
    hkwt = nc.dram_tensor("hkwt", [24, 128, DC, 256], bf16, kind="ExternalInput")
    wupt = nc.dram_tensor("wupt", [48, 128, DC, 128], bf16, kind="ExternalInput")
    wdnt = nc.dram_tensor("wdnt", [DC, 128, IC, 128], bf16, kind="ExternalInput")
    wmpt = nc.dram_tensor("wmpt", [128, DC, D], bf16, kind="ExternalInput")
    dw = nc.dram_tensor("dw", [128, IC, 3], f32, kind="ExternalInput")
    bqk = nc.dram_tensor("bqk", [DS, 2], f32, kind="ExternalInput")
    bhot = nc.dram_tensor("bhot", [128, 4], f32, kind="ExternalInput")
    hmask = nc.dram_tensor("hmask", [128, 2], f32, kind="ExternalInput")
    rows = nc.dram_tensor("rows", [1, 5 * D], f32, kind="ExternalInput")
    out_e = nc.dram_tensor("out", [NT, D], f32, kind="ExternalOutput")

    ident_f32_d = nc.inline_tensor(np.eye(128, dtype=np.float32), name="idf32")
    import ml_dtypes
    ident_bf16_d = nc.inline_tensor(
        np.eye(128).astype(ml_dtypes.bfloat16), name="idbf16"
    )

    # collective bounce buffers + scratch (internal DRAM)
    qp_i = nc.dram_tensor("qp_i", [128, DC, 4], f32)
    qp_o = nc.dram_tensor("qp_o", [128, DC, 4], f32, addr_space="Shared")
    og_i = nc.dram_tensor("og_i", [RSH, 8], f32)
    og_o = nc.dram_tensor("og_o", [8 * RSH, 8], f32, addr_space="Shared")
    ce_i = nc.dram_tensor("ce_i", [DS, 772], f32)
    ce_o = nc.dram_tensor("ce_o", [DS, 772], f32)
    hx_i = nc.dram_tensor("hx_i", [128, 2, IC], f32)
    hx_o = nc.dram_tensor("hx_o", [2, 128, 2, IC], f32)
    qint_d = nc.dram_tensor("qint_d", [NT, D], f32)
    hf_d = nc.dram_tensor("hf_d", [INNER, NT], bf16)

    RG_ALL = [list(range(8))]
    RG_PAIR = [[0, 1], [2, 3], [4, 5], [6, 7]]

    with tile.TileContext(nc) as tc:
        import contextlib

        ctx = contextlib.ExitStack()
        with ctx:
            cst = ctx.enter_context(tc.tile_pool(name="cst", bufs=1))
            big_a = ctx.enter_context(tc.tile_pool(name="big_a", bufs=1))
            big_b = ctx.enter_context(tc.tile_pool(name="big_b", bufs=1))
            med = ctx.enter_context(tc.tile_pool(name="med", bufs=1))
            sml = ctx.enter_context(tc.tile_pool(name="sml", bufs=2))
            scr = ctx.enter_context(tc.tile_pool(name="scr", bufs=3))
            wst = ctx.enter_context(tc.tile_pool(name="wst", bufs=2))
            wst4 = ctx.enter_context(tc.tile_pool(name="wst4", bufs=3))
            ps = ctx.enter_context(tc.tile_pool(name="ps", bufs=2, space="PSUM"))
            ps_tp = ctx.enter_context(
                tc.tile_pool(name="ps_tp", bufs=2, space="PSUM")
            )
            ps_acc = ctx.enter_context(
                tc.tile_pool(name="ps_acc", bufs=1, space="PSUM")
            )

            # ---- constants to SBUF ----
            id32 = cst.tile([128, 128], f32, tag="id32")
            nc.sync.dma_start(id32[:], ident_f32_d[:])
            id16 = cst.tile([128, 128], bf16, tag="id16")
            nc.sync.dma_start(id16[:], ident_bf16_d[:])
            dw_sb = cst.tile([128, IC, 3], f32, tag="dw")
            nc.sync.dma_start(dw_sb[:], dw[:])
            bqk_sb = cst.tile([DS, 2], f32, tag="bqk")
            nc.sync.dma_start(bqk_sb[:], bqk[:])
            bhot_sb = cst.tile([128, 4], f32, tag="bhot")
            nc.sync.dma_start(bhot_sb[:], bhot[:])
            hmask_sb = cst.tile([128, 2], f32, tag="hmask")
            nc.sync.dma_start(hmask_sb[:], hmask[:])
            rows_sb = cst.tile([1, 5 * D], f32, tag="rows")
            nc.sync.dma_start(rows_sb[:], rows[:])
            zero1 = cst.tile([128, 1], f32, tag="zero1")
            nc.vector.memset(zero1[:], 0.0)
            eps1 = cst.tile([128, 1], f32, tag="eps1")
            nc.vector.memset(eps1[:], EPS)
            nc.const_aps.aps[(f32, 0.0)] = zero1[:]
            nc.const_aps.aps[(f32, EPS)] = eps1[:]

            def row_bc(i):  # [1,768] -> broadcast [128,768]
                return rows_sb[0:1, i * D:(i + 1) * D].partition_broadcast(128)

            # =========================================================
            # Phase A: LN1 + H (token-major bf16), transpose -> H_d, q_pool
            # =========================================================
            h_tok = big_a.tile([128, TC, D], bf16, tag="bigA")
            h_d = big_b.tile([128, DC, NT], bf16, tag="h_d")
            qs2 = sml.tile([128, DC, 2], f32, tag="qs2")

            def layernorm_chunk(src_ap, out_ap, gcol, bcol, use_gb, extra_add):
                """LN over free dim of [128,768] chunk; out = LN*g+b (+extra)."""
                mv6 = scr.tile([128, 2, 6], f32, tag="mv6")
                for g in range(2):
                    nc.vector.bn_stats(
                        mv6[:, g, :], src_ap[:, g * 384:(g + 1) * 384]
                    )
                mv = scr.tile([128, 2], f32, tag="mv")
                nc.vector.bn_aggr(mv[:], mv6[:])
                rs = scr.tile([128, 1], f32, tag="rs")
                sd = scr.tile([128, 1], f32, tag="sd")
                nc.scalar.activation(sd[:], mv[:, 1:2], AF.Sqrt, bias=EPS)
                nc.vector.reciprocal(rs[:], sd[:])
                xn = scr.tile([128, D], f32, tag="t768")
                nc.vector.tensor_scalar(
                    xn[:], src_ap, mv[:, 0:1], rs[:, 0:1], ALU.subtract, ALU.mult
                )
                if use_gb:
                    nc.vector.tensor_tensor(xn[:], xn[:], row_bc(gcol), ALU.mult)
                    nc.vector.tensor_tensor(xn[:], xn[:], row_bc(bcol), ALU.add)
                if extra_add is not None:
                    nc.vector.tensor_tensor(out_ap, xn[:], extra_add, ALU.add)
                else:
                    nc.vector.tensor_copy(out_ap, xn[:])

            for t in range(TC):
                qc = sml.tile([128, D], f32, tag="qc")
                nc.sync.dma_start(qc[:], q_in[t * 128:(t + 1) * 128, :])
                xc = sml.tile([128, D], f32, tag="xc")
                nc.sync.dma_start(xc[:], x_in[t * 128:(t + 1) * 128, :])
                layernorm_chunk(qc[:], h_tok[:, t, :], 0, 1, use_ln1_gb, xc[:])

            # transpose H -> H_d (bf16) + q_pool partials via accum_out
            for dc in range(DC):
                for tq in range(2):
                    ptp = ps_tp.tile([128, 512], bf16, tag="tp")
                    for k in range(4):
                        t = tq * 4 + k
                        nc.tensor.matmul(
                            ptp[:, k * 128:(k + 1) * 128],
                            h_tok[:, t, dc * 128:(dc + 1) * 128],
                            id16[:],
                            is_transpose=True,
                        )
                    nc.scalar.activation(
                        h_d[:, dc, tq * 512:(tq + 1) * 512],
                        ptp[:],
                        AF.Copy,
                        accum_out=qs2[:, dc, tq:tq + 1],
                    )

            # qp contribution: [128, DC, 4] = bhot * (qs2.sum) / N
            qp_sb = sml.tile([128, DC, 4], f32, tag="qp_sb")
            qsum = sml.tile([128, DC], f32, tag="qsum")
            nc.vector.tensor_tensor(qsum[:], qs2[:, :, 0], qs2[:, :, 1], ALU.add)
            for dc in range(DC):
                nc.vector.tensor_scalar(
                    qp_sb[:, dc, :],
                    bhot_sb[:],
                    qsum[:, dc:dc + 1],
                    1.0 / N,
                    ALU.mult,
                    ALU.mult,
                )
            nc.gpsimd.dma_start(qp_i[:], qp_sb[:])
            nc.gpsimd.collective_compute(
                "AllReduce", ALU.add, replica_groups=RG_ALL,
                ins=[qp_i[:]], outs=[qp_o[:]],
            )
            qp_all = sml.tile([128, DC, 4], f32, tag="qp_all")
            nc.gpsimd.dma_start(qp_all[:], qp_o[:])
            qp_bf = sml.tile([128, DC, 4], bf16, tag="qp_bf")
            nc.vector.tensor_copy(qp_bf[:], qp_all[:])

            # =========================================================
            # Phase B: hyper O_part [RSH, 8] -> AllGather -> Omega (bf16)
            # =========================================================
            for rb in range(RSH // 256):
                obs = []
                for m, wsrc in ((0, hqwt), (1, hkwt)):
                    wblk = wst.tile([128, DC, 256], bf16, tag="hypW")
                    nc.sync.dma_start(wblk[:], wsrc[rb])
                    po = ps_acc.tile([4, 256], f32, tag="acc")
                    for j in range(DC):
                        nc.tensor.matmul(
                            po[:],
                            qp_bf[:, j, :],
                            wblk[:, j, :],
                            start=(j == 0),
                            stop=(j == DC - 1),
                        )
                    ob = scr.tile([4, 256], f32, tag="og_blk")
                    nc.scalar.activation(ob[:], po[:], AF.Copy)
                    obs.append(ob)
                for half in range(2):
                    obt = scr.tile([128, 8], f32, tag="obt")
                    for m in range(2):
                        ptpo = ps_tp.tile([128, 4], f32, tag="tp")
                        nc.tensor.matmul(
                            ptpo[:],
                            obs[m][:, half * 128:(half + 1) * 128],
                            id32[:4, :4],
                            is_transpose=True,
                        )
                        nc.scalar.activation(
                            obt[:, m * 4:(m + 1) * 4], ptpo[:], AF.Copy
                        )
                    nc.sync.dma_start(
                        og_i[rb * 256 + half * 128:
                             rb * 256 + (half + 1) * 128, :],
                        obt[:],
                    )
            nc.gpsimd.collective_compute(
                "AllGather", ALU.bypass, replica_groups=RG_ALL,
                ins=[og_i[:]], outs=[og_o[:]],
            )
            # Omega: mask-reduce gathered columns by bhot (SPMD-safe), bf16
            om = []
            for m in range(2):
                omt = med.tile([128, DC, DS], bf16, tag=f"om{m}")
                om.append(omt)
            for dc in range(DC):
                og8 = sml.tile([128, DS, 8], f32, tag="og8")
                nc.sync.dma_start(
                    og8[:],
                    og_o[:].rearrange(
                        "(c p s) e -> p c s e", p=128, s=DS
                    )[:, dc, :, :],
                )
                for m in range(2):
                    acc4 = scr.tile([128, DS], f32, tag="omacc")
                    t4 = scr.tile([128, DS], f32, tag="omtmp")
                    nc.vector.tensor_scalar(
                        acc4[:], og8[:, :, m * 4], bhot_sb[:, 0:1], None,
                        ALU.mult,
                    )
                    for bb in range(1, 4):
                        nc.vector.tensor_scalar(
                            t4[:], og8[:, :, m * 4 + bb], bhot_sb[:, bb:bb + 1],
                            None, ALU.mult,
                        )
                        nc.vector.tensor_tensor(acc4[:], acc4[:], t4[:], ALU.add)
                    nc.vector.tensor_copy(om[m][:, dc, :], acc4[:])

            # =========================================================
            # Phase C: attention  Phi -> C -> Attraction -> m -> m_proj
            # =========================================================
            phi = []
            for m in range(2):
                phi_t = med.tile([DS, NT], bf16, tag=f"phi{m}")
                for tq in range(2):
                    pph = ps_acc.tile([DS, 512], f32, tag="acc")
                    for dc in range(DC):
                        nc.tensor.matmul(
                            pph[:],
                            om[m][:, dc, :],
                            h_d[:, dc, tq * 512:(tq + 1) * 512],
                            start=(dc == 0),
                            stop=(dc == DC - 1),
                        )
                    # elu(x+b)+1 = exp(u - relu(u)) + relu(u),  u = x + bias
                    rl = sml.tile([DS, 512], f32, tag="rl")
                    nc.scalar.activation(
                        rl[:], pph[:], AF.Relu, bias=bqk_sb[:, m:m + 1]
                    )
                    u = sml.tile([DS, 512], f32, tag="u")
                    nc.vector.tensor_scalar(
                        u[:], pph[:], bqk_sb[:, m:m + 1], None, ALU.add
                    )
                    nc.vector.tensor_tensor(u[:], u[:], rl[:], ALU.subtract)
                    ex = sml.tile([DS, 512], f32, tag="ex")
                    nc.scalar.activation(ex[:], u[:], AF.Exp)
                    nc.vector.tensor_tensor(
                        phi_t[:, tq * 512:(tq + 1) * 512], ex[:], rl[:], ALU.add
                    )
                phi.append(phi_t)

            # phi_k_sum [DS,1]
            pks = sml.tile([DS, 1], f32, tag="pks")
            nc.vector.tensor_reduce(pks[:], phi[1][:], AX.X, ALU.add)

            # Phi_K token-major bf16 [128, TC, DS]
            pk_tok = med.tile([128, TC, DS], bf16, tag="pk_tok")
            for tq in range(2):
                ptp = ps_tp.tile([128, 256], bf16, tag="tp")
                for k in range(4):
                    t = tq * 4 + k
                    nc.tensor.matmul(
                        ptp[:, k * DS:(k + 1) * DS],
                        phi[1][:, t * 128:(t + 1) * 128],
                        id16[:DS, :DS],
                        is_transpose=True,
                    )
                nc.scalar.activation(
                    pk_tok[:, tq * 4:(tq + 1) * 4, :]
                    .rearrange("p t s -> p (t s)"),
                    ptp[:],
                    AF.Copy,
                )

            # C partial [DS, 772]: cols 0:768 = C, 768 = pks
            ce_sb = med.tile([DS, 772], f32, tag="ce_sb")
            for off, w in ((0, 512), (512, 256)):
                pc0 = ps_acc.tile([DS, w], f32, tag="acc")
                for t in range(TC):
                    nc.tensor.matmul(
                        pc0[:],
                        pk_tok[:, t, :],
                        h_tok[:, t, off:off + w],
                        start=(t == 0),
                        stop=(t == TC - 1),
                    )
                nc.scalar.activation(ce_sb[:, off:off + w], pc0[:], AF.Copy)
            nc.vector.tensor_copy(ce_sb[:, 768:769], pks[:])
            nc.vector.memset(ce_sb[:, 769:772], 0.0)
            nc.gpsimd.dma_start(ce_i[:], ce_sb[:])
            nc.gpsimd.collective_compute(
                "AllReduce", ALU.add, replica_groups=RG_PAIR,
                ins=[ce_i[:]], outs=[ce_o[:]],
            )
            ce_all = med.tile([DS, 772], bf16, tag="ce_a16")
            nc.gpsimd.dma_start(ce_all[:], ce_o[:])

            # Attraction + m (token-major), m transposed -> m_d (bf16)
            m_d = med.tile([128, DC, NT], bf16, tag="mdq")
            for t in range(TC):
                pat = ps_acc.tile([128, 772], f32, tag="acc")
                nc.tensor.matmul(
                    pat[:, 0:512],
                    phi[0][:, t * 128:(t + 1) * 128],
                    ce_all[:, 0:512],
                )
                nc.tensor.matmul(
                    pat[:, 512:772],
                    phi[0][:, t * 128:(t + 1) * 128],
                    ce_all[:, 512:772],
                )
                dn = scr.tile([128, 1], f32, tag="dn")
                nc.scalar.activation(dn[:], pat[:, 768:769], AF.Abs)
                nc.vector.tensor_scalar(dn[:], dn[:], 1.0, None, ALU.add)
                rn = scr.tile([128, 1], f32, tag="rn")
                nc.vector.reciprocal(rn[:], dn[:])
                mm = scr.tile([128, D], f32, tag="t768")
                nc.vector.tensor_scalar(
                    mm[:], pat[:, 0:768], rn[:, 0:1], None, ALU.mult
                )
                nc.vector.tensor_tensor(mm[:], mm[:], h_tok[:, t, :], ALU.subtract)
                for dc in range(DC):
                    ptpm = ps_tp.tile([128, 128], f32, tag="tp")
                    nc.tensor.matmul(
                        ptpm[:],
                        mm[:, dc * 128:(dc + 1) * 128],
                        id32[:],
                        is_transpose=True,
                    )
                    nc.scalar.activation(
                        m_d[:, dc, t * 128:(t + 1) * 128], ptpm[:], AF.Copy
                    )

            # m_proj weights (bf16, [in,out], resident)
            wmp_sb = med.tile([128, DC, D], bf16, tag="wmpT")
            nc.sync.dma_start(wmp_sb[:], wmpt[:])

            # m_proj -> Q_interact (-> DRAM) -> LN2 -> qn2_d (bf16)
            qn2_d = med.tile([128, DC, NT], bf16, tag="mdq2")
            for t in range(TC):
                pmp = ps_acc.tile([128, 768], f32, tag="acc")
                for half, w in ((0, 512), (1, 256)):
                    for dc in range(DC):
                        nc.tensor.matmul(
                            pmp[:, half * 512:half * 512 + w],
                            m_d[:, dc, t * 128:(t + 1) * 128],
                            wmp_sb[:, dc, half * 512:half * 512 + w],
                            start=(dc == 0),
                            stop=(dc == DC - 1),
                        )
                qi = scr.tile([128, D], f32, tag="t768")
                nc.scalar.activation(qi[:], pmp[:], AF.Copy, scale=sp_val)
                qc2 = scr.tile([128, D], f32, tag="t768")
                nc.sync.dma_start(qc2[:], q_in[t * 128:(t + 1) * 128, :])
                nc.vector.tensor_tensor(qi[:], qi[:], qc2[:], ALU.add)
                if use_mpb:
                    nc.vector.tensor_tensor(qi[:], qi[:], row_bc(4), ALU.add)
                nc.sync.dma_start(qint_d[t * 128:(t + 1) * 128, :], qi[:])
                qn = scr.tile([128, D], f32, tag="t768")
                layernorm_chunk(qi[:], qn[:], 2, 3, use_ln2_gb, None)
                for dc in range(DC):
                    ptpm = ps_tp.tile([128, 128], f32, tag="tp")
                    nc.tensor.matmul(
                        ptpm[:],
                        qn[:, dc * 128:(dc + 1) * 128],
                        id32[:],
                        is_transpose=True,
                    )
                    nc.scalar.activation(
                        qn2_d[:, dc, t * 128:(t + 1) * 128], ptpm[:], AF.Copy
                    )

            # =========================================================
            # Phase D: FFN  GU -> silu*U -> hf (DRAM) -> conv -> W_down
            # =========================================================
            hx_sb = sml.tile([128, 2, IC], f32, tag="hx_sb")
            for ic in range(IC):
                wtg = wst4.tile([128, DC, 128], bf16, tag="wupT")
                nc.sync.dma_start(wtg[:], wupt[ic])
                wtu = wst4.tile([128, DC, 128], bf16, tag="wupT")
                nc.sync.dma_start(wtu[:], wupt[ic + IC])
                hf = sml.tile([128, NT], bf16, tag="hf")
                for tq in range(2):
                    pg = ps.tile([128, 512], f32, tag="gup")
                    pu = ps.tile([128, 512], f32, tag="guu")
                    for wt, pdst in ((wtg, pg), (wtu, pu)):
                        for dc in range(DC):
                            nc.tensor.matmul(
                                pdst[:],
                                wt[:, dc, :],
                                qn2_d[:, dc, tq * 512:(tq + 1) * 512],
                                start=(dc == 0),
                                stop=(dc == DC - 1),
                            )
                    sgh = sml.tile([128, 512], bf16, tag="sg")
                    nc.scalar.activation(sgh[:], pg[:], AF.Silu)
                    nc.vector.tensor_tensor(
                        hf[:, tq * 512:(tq + 1) * 512], sgh[:], pu[:], ALU.mult
                    )
                nc.vector.tensor_copy(hx_sb[:, 0, ic:ic + 1], hf[:, 0:1])
                nc.vector.tensor_copy(hx_sb[:, 1, ic:ic + 1], hf[:, NT - 1:NT])
                nc.sync.dma_start(hf_d[ic * 128:(ic + 1) * 128, :], hf[:])

            # halo exchange (pair AllGather)
            nc.gpsimd.dma_start(hx_i[:], hx_sb[:])
            nc.gpsimd.collective_compute(
                "AllGather", ALU.bypass, replica_groups=RG_PAIR,
                ins=[hx_i[:]], outs=[hx_o[:]],
            )
            hxn = sml.tile([128, 2, IC], f32, tag="hxn")
            # hxn[:,0,:] = partner-left-candidate = block0 right boundary
            # hxn[:,1,:] = partner-right-candidate ... masks pick the valid one
            nc.gpsimd.dma_start(hxn[:, 0, :], hx_o[0, :, 1, :])
            nc.gpsimd.dma_start(hxn[:, 1, :], hx_o[1, :, 0, :])
            halo = sml.tile([128, IC, 2], f32, tag="halo")
            for k in range(2):
                nc.vector.tensor_scalar(
                    halo[:, :, k], hxn[:, k, :], hmask_sb[:, k:k + 1], None,
                    ALU.mult,
                )

            # conv (reload hf with halo) -> co (SBUF, bf16, full)
            co = big_a.tile([128, IC, NT], bf16, tag="bigA")
            for ic in range(IC):
                hfr = sml.tile([128, NT + 2], bf16, tag="hfr")
                nc.sync.dma_start(
                    hfr[:, 1:NT + 1], hf_d[ic * 128:(ic + 1) * 128, :]
                )
                nc.vector.tensor_copy(hfr[:, 0:1], halo[:, ic, 0:1])
                nc.vector.tensor_copy(hfr[:, NT + 1:NT + 2], halo[:, ic, 1:2])
                t0 = sml.tile([128, NT], bf16, tag="cv0")
                nc.vector.tensor_scalar(
                    t0[:], hfr[:, 0:NT], dw_sb[:, ic, 0:1], None, ALU.mult
                )
                t1 = sml.tile([128, NT], bf16, tag="cv1")
                nc.vector.tensor_scalar(
                    t1[:], hfr[:, 1:NT + 1], dw_sb[:, ic, 1:2], None, ALU.mult
                )
                nc.vector.tensor_tensor(t0[:], t0[:], t1[:], ALU.add)
                t2 = sml.tile([128, NT], bf16, tag="cv1")
                nc.vector.tensor_scalar(
                    t2[:], hfr[:, 2:NT + 2], dw_sb[:, ic, 2:3], None, ALU.mult
                )
                nc.vector.tensor_tensor(co[:, ic, :], t0[:], t2[:], ALU.add)

            # W_down: H_out (d-major) -> transpose -> + Q_int -> store
            ho_d = med.tile([128, DC, NT], bf16, tag="mdq")
            for oc in range(DC):
                wd = wst.tile([128, IC, 128], bf16, tag="wdnT")
                nc.sync.dma_start(wd[:], wdnt[oc])
                pho = ps_acc.tile([128, NT], f32, tag="acc")
                for tq in range(2):
                    for ic in range(IC):
                        nc.tensor.matmul(
                            pho[:, tq * 512:(tq + 1) * 512],
                            wd[:, ic, :],
                            co[:, ic, tq * 512:(tq + 1) * 512],
                            start=(ic == 0),
                            stop=(ic == IC - 1),
                        )
                nc.scalar.activation(ho_d[:, oc, :], pho[:], AF.Copy)
            for t in range(TC):
                ptp = ps_tp.tile([128, 768], bf16, tag="tp")
                for dc in range(DC):
                    nc.tensor.matmul(
                        ptp[:, dc * 128:(dc + 1) * 128],
                        ho_d[:, dc, t * 128:(t + 1) * 128],
                        id16[:],
                        is_transpose=True,
                    )
                qr = scr.tile([128, D], f32, tag="t768")
                nc.sync.dma_start(qr[:], qint_d[t * 128:(t + 1) * 128, :])
                ot = scr.tile([128, D], f32, tag="t768")
                nc.vector.tensor_tensor(ot[:], ptp[:], qr[:], ALU.add)
                nc.sync.dma_start(out_e[t * 128:(t + 1) * 128, :], ot[:])

    nc.finalize()
    return nc


def _install_trace_hook():
    import types
    import antenv
    if "antenv.axon_hooks" in sys.modules:
        return
    mod = types.ModuleType("antenv.axon_hooks")
    _h = [None]
    mod.set_axon_ntff_profile_hook = lambda h: _h.__setitem__(0, h)
    mod.get_axon_ntff_profile_hook = lambda: _h[0]
    sys.modules["antenv.axon_hooks"] = mod
    antenv.axon_hooks = mod
    try:
        from trn_agent_boot.trn_boot import _ntff_profile_via_ctypes
        mod.set_axon_ntff_profile_hook(
            _ntff_profile_via_ctypes("/opt/axon/libaxon_pjrt.so")
        )
    except Exception as e:
        print(f"ntff hook install failed: {e}")


def kernel(**inputs):
    from concourse.bass_utils import run_bass_kernel_spmd
    import ml_dtypes

    if TRACE:
        _install_trace_hook()

    bfnp = ml_dtypes.bfloat16

    Q_in = np.asarray(inputs["Q_in"], np.float32)
    X = np.asarray(inputs["X"], np.float32)
    dt = float(np.asarray(inputs["dt"]))
    hyper_q_w = np.asarray(inputs["hyper_q_w"], np.float32)
    hyper_k_w = np.asarray(inputs["hyper_k_w"], np.float32)
    B_Q = np.asarray(inputs["B_Q"], np.float32)
    B_K = np.asarray(inputs["B_K"], np.float32)
    m_proj_w = np.asarray(inputs["m_proj_w"], np.float32)
    m_proj_b = np.asarray(inputs["m_proj_b"], np.float32)
    n1g = np.asarray(inputs["norm1_g"], np.float32)
    n1b = np.asarray(inputs["norm1_b"], np.float32)
    n2g = np.asarray(inputs["norm2_g"], np.float32)
    n2b = np.asarray(inputs["norm2_b"], np.float32)
    W_up_w = np.asarray(inputs["W_up_w"], np.float32)
    dw_w = np.asarray(inputs["dw_w"], np.float32)
    W_down_w = np.asarray(inputs["W_down_w"], np.float32)

    sp = float(np.log1p(np.exp(dt)))
    use_ln1_gb = not (np.allclose(n1g, 1.0) and np.allclose(n1b, 0.0))
    use_ln2_gb = not (np.allclose(n2g, 1.0) and np.allclose(n2b, 0.0))
    use_mpb = not np.allclose(m_proj_b, 0.0)

    key = (sp, use_ln1_gb, use_ln2_gb, use_mpb)
    if key not in _CACHE:
        _CACHE[key] = _build(sp, use_ln1_gb, use_ln2_gb, use_mpb)
    nc = _CACHE[key]

    # host-side weight prep (transpose + bf16 cast + tile-major layout)
    # wupt[oc] = [128(p), DC, 128(o)]: W_up.T[in, out] tile, p-major
    wupt_h = np.ascontiguousarray(
        W_up_w.T.astype(bfnp).reshape(DC, 128, 48, 128).transpose(2, 1, 0, 3)
    )
    wdnt_h = np.ascontiguousarray(
        W_down_w.T.astype(bfnp).reshape(IC, 128, DC, 128).transpose(2, 1, 0, 3)
    )
    wmpt_h = np.ascontiguousarray(
        m_proj_w.T.astype(bfnp).reshape(DC, 128, D).transpose(1, 0, 2)
    )
    # hyper W.T per core, r-block-major: [24(rb), 128(p=j%128), DC(j//128), 256(r)]
    hqwt_full = hyper_q_w.astype(bfnp).T.reshape(DC, 128, 8, 24, 256)
    hkwt_full = hyper_k_w.astype(bfnp).T.reshape(DC, 128, 8, 24, 256)

    dw_dev = np.ascontiguousarray(
        dw_w[:, 0, :].reshape(IC, 128, 3).transpose(1, 0, 2)
    )
    bqk_dev = np.stack([B_Q, B_K], axis=1)
    rows_dev = np.ascontiguousarray(
        np.concatenate([n1g, n1b, n2g, n2b, sp * m_proj_b])[None, :]
    )

    in_maps = []
    for c in range(8):
        b, h = divmod(c, 2)
        bhot_dev = np.zeros((128, 4), np.float32)
        bhot_dev[:, b] = 1.0
        hmask_dev = np.zeros((128, 2), np.float32)
        hmask_dev[:, 0] = 1.0 if h == 1 else 0.0
        hmask_dev[:, 1] = 1.0 if h == 0 else 0.0
        in_maps.append({
            "q_in": np.ascontiguousarray(Q_in[b, h * NT:(h + 1) * NT]),
            "x_in": np.ascontiguousarray(X[b, h * NT:(h + 1) * NT]),
            "hqwt": np.ascontiguousarray(
                hqwt_full[:, :, c].transpose(2, 1, 0, 3)),
            "hkwt": np.ascontiguousarray(
                hkwt_full[:, :, c].transpose(2, 1, 0, 3)),
            "wupt": wupt_h,
            "wdnt": wdnt_h,
            "wmpt": wmpt_h,
            "dw": dw_dev,
            "bqk": bqk_dev,
            "bhot": bhot_dev,
            "hmask": hmask_dev,
            "rows": rows_dev,
        })

    global _LAST_RESULT
    res = run_bass_kernel_spmd(nc, in_maps, list(range(8)), trace=TRACE)
    _LAST_RESULT = res
    out = np.empty((B, N, D), np.float32)
    for c in range(8):
        b, h = divmod(c, 2)
        out[b, h * NT:(h + 1) * NT] = res.results[c]["out"]
    return out


if __name__ == "__main__":
    rng = np.random.default_rng(0)
    ins = {
        "Q_in": rng.normal(size=(B, N, D)).astype(np.float32),
        "X": rng.normal(size=(B, N, D)).astype(np.float32),
        "dt": np.float32(0.1),
        "hyper_q_w": rng.normal(size=(D * DS, D)).astype(np.float32) / 27.7,
        "hyper_k_w": rng.normal(size=(D * DS, D)).astype(np.float32) / 27.7,
        "B_Q": rng.normal(size=(DS,)).astype(np.float32) * 0.02,
        "B_K": rng.normal(size=(DS,)).astype(np.float32) * 0.02,
        "m_proj_w": rng.normal(size=(D, D)).astype(np.float32) * 0.02,
        "m_proj_b": np.zeros((D,), np.float32),
        "norm1_g": np.ones((D,), np.float32),
        "norm1_b": np.zeros((D,), np.float32),
        "norm2_g": np.ones((D,), np.float32),
        "norm2_b": np.zeros((D,), np.float32),
        "W_up_w": rng.normal(size=(2 * INNER, D)).astype(np.float32) / 27.7,
        "dw_w": rng.normal(size=(INNER, 1, 3)).astype(np.float32) / 1.7,
        "W_down_w": rng.normal(size=(D, INNER)).astype(np.float32) / 55.4,
    }
    out = kernel(**ins)
    print("out", out.shape, out.dtype, np.abs(out).mean())
